# revision 1
# baseline (speedup 1.0000x reference)
"""Trainium2 Bass kernel for the proxy-NCA-style Criterion loss.

Math (verified exactly equivalent to the reference):
  bn = normalize(batch, dim=1); pn = normalize(proxies, dim=1)
  sims[i,c] = bn[i] . pn[c]
  d[i] = sims[i, labels[i]]              (diagonal)
  neg branch: s_neg[c] = sum_i exp(32*sims[i,c] + 3.2) - corr[c]
              corr[c]  = sum_{i: labels[i]=c} exp(32*d[i] + 3.2)
              neg_s[c] = softplus(logsumexp) = log1p(s_neg[c])
  pos branch: columns j with equal labels are identical;
              s_pos[j] = t[labels[j]],  t[k] = sum_{i: labels[i]=k} exp(-32*d[i] + 3.2)
              pos_s[j] = log1p(s_pos[j])
  loss = mean(neg_s) + mean(pos_s)
  (The reference's nz masks are all-True for this problem's input regime --
  verified against the reference: every column has at least one unmasked
  entry and max+min of the masked column is never exactly 0.)

Device work (8 cores, class-sharded): the big [4096 x 16384] similarity
matmul fused with exp and column-sum (ACT accum_out), plus the diagonal
row-dots.  Host work: input normalization/transposes (sharding prep) and
the O(BS + C) scatter-add / log1p / mean combine (the gather/all-reduce).
"""

import numpy as np

BS, C, D = 4096, 16384, 128
NCORES = 8
CS = C // NCORES          # 2048 classes per core
BSH = BS // NCORES        # 512 batch rows per core (diagonal shard)
CT = 128                  # classes per tile (PSUM partitions)
IG = 2048                 # batch columns per ACT group (4 PSUM banks)
NCT = CS // CT            # 16 class tiles per core
NIG = BS // IG            # 2 i-groups
NMM = IG // 512           # 4 matmuls per group
NDT = BSH // CT           # 4 diagonal tiles per core

_NC_CACHE = []
LAST_RESULTS = None       # test.py reads exec_time_ns from here


def _build_nc(repeat=1):
    import concourse.bacc as bacc
    import concourse.mybir as mybir
    from concourse import tile

    fp32 = mybir.dt.float32
    # float32r: fp32 matmul variant that streams at 1 cycle/row (vs 4 for
    # plain fp32) on TRN2; numerically verified against the f64 oracle.
    fp32r = mybir.dt.float32r
    nc = bacc.Bacc(None)

    bT = nc.declare_dram_parameter("bT", [D, BS], fp32r, isOutput=False)
    pT = nc.declare_dram_parameter("pT", [D, CS], fp32r, isOutput=False)
    bg = nc.declare_dram_parameter("bg", [BSH, 2 * D], fp32, isOutput=False)
    colsum = nc.declare_dram_parameter("colsum", [CT, NCT], fp32, isOutput=True)
    dpart = nc.declare_dram_parameter("dpart", [CT, NDT], fp32, isOutput=True)

    with tile.TileContext(nc) as tc:
        with (
            tc.tile_pool(name="big", bufs=1) as big,
            tc.tile_pool(name="work", bufs=3) as work,
            tc.tile_pool(name="psum", bufs=2, space="PSUM") as psum,
        ):
            bT_t = big.tile([D, BS], fp32r)
            pT_t = big.tile([D, CS], fp32r)
            # chunked loads so multiple DMA queues run in parallel; the
            # first pT chunk and first bT chunk go out first so the first
            # class-tile's matmuls can start as early as possible.
            nc.sync.dma_start(pT_t[:, 0:512], pT[:, 0:512])
            for j in range(8):
                nc.sync.dma_start(
                    bT_t[:, j * 512 : (j + 1) * 512], bT[:, j * 512 : (j + 1) * 512]
                )
            for j in range(1, 4):
                nc.sync.dma_start(
                    pT_t[:, j * 512 : (j + 1) * 512], pT[:, j * 512 : (j + 1) * 512]
                )

            bias_t = big.tile([CT, 1], fp32)
            nc.vector.memset(bias_t[:], 3.2)

            bg_all = big.tile([CT, NDT * 2 * D], fp32)
            nc.sync.dma_start(
                bg_all[:, :].rearrange("p (t d) -> p t d", t=NDT),
                bg[:, :].rearrange("(t p) d -> p t d", p=CT),
            )

            acc = big.tile([CT, NIG * NCT], fp32)    # [p, g*NCT+ct]
            cs_t = big.tile([CT, NCT], fp32)
            d_t = big.tile([CT, NDT], fp32)

            for _r in range(repeat):
                for ct in range(NCT):
                    for g in range(NIG):
                        ps = psum.tile([CT, IG], fp32, tag="ps")
                        for j in range(NMM):
                            nc.tensor.matmul(
                                ps[:, j * 512 : (j + 1) * 512],
                                pT_t[:, ct * CT : (ct + 1) * CT],
                                bT_t[:, g * IG + j * 512 : g * IG + (j + 1) * 512],
                                start=True,
                                stop=True,
                            )
                        # exp(32*sims + 3.2) fused with the column-sum
                        # (accum_out); output written back over the PSUM
                        # tile in place -- the full tile is dead after the
                        # accumulated sum is extracted.
                        nc.scalar.activation(
                            ps[:],
                            ps[:],
                            mybir.ActivationFunctionType.Exp,
                            bias=bias_t[:],
                            scale=32.0,
                            accum_out=acc[:, g * NCT + ct : g * NCT + ct + 1],
                        )

                for t in range(NDT):
                    sc2 = work.tile([CT, D], fp32, tag="sc2")
                    nc.vector.scalar_tensor_tensor(
                        sc2[:],
                        bg_all[:, t * 2 * D : t * 2 * D + D],
                        1.0,
                        bg_all[:, t * 2 * D + D : (t + 1) * 2 * D],
                        mybir.AluOpType.mult,
                        mybir.AluOpType.mult,
                        accum_out=d_t[:, t : t + 1],
                    )

            nc.vector.tensor_add(cs_t[:], acc[:, 0:NCT], acc[:, NCT : 2 * NCT])
            nc.gpsimd.dma_start(colsum[:, :], cs_t[:])
            nc.gpsimd.dma_start(dpart[:, :], d_t[:])

    nc.compile()
    return nc


def kernel(batch, proxies, labels):
    global LAST_RESULTS
    from concourse.bass_utils import run_bass_kernel_spmd

    batch = np.asarray(batch, dtype=np.float32)
    proxies = np.asarray(proxies, dtype=np.float32)
    lab = np.asarray(labels).astype(np.int64)

    bn = batch / np.linalg.norm(batch, axis=1, keepdims=True).astype(np.float32)
    pn = proxies / np.linalg.norm(proxies, axis=1, keepdims=True).astype(np.float32)
    gath = pn[lab]                                  # [BS, D] proxies of own label

    bT = np.ascontiguousarray(bn.T)                 # [D, BS]
    in_maps = []
    for k in range(NCORES):
        in_maps.append(
            {
                "bT": bT,
                "pT": np.ascontiguousarray(pn[k * CS : (k + 1) * CS].T),
                "bg": np.ascontiguousarray(
                    np.concatenate(
                        [
                            bn[k * BSH : (k + 1) * BSH],
                            gath[k * BSH : (k + 1) * BSH],
                        ],
                        axis=1,
                    )
                ),
            }
        )

    if not _NC_CACHE:
        _NC_CACHE.append(_build_nc())
    nc = _NC_CACHE[0]

    LAST_RESULTS = run_bass_kernel_spmd(nc, in_maps, list(range(NCORES)))
    res = LAST_RESULTS.results

    colsum = np.empty(C, np.float64)
    d = np.empty(BS, np.float64)
    for k in range(NCORES):
        cs = res[k]["colsum"].astype(np.float64)    # [CT, NCT]; class = ct*CT + p
        colsum[k * CS : (k + 1) * CS] = cs.T.reshape(-1)
        dp = res[k]["dpart"].astype(np.float64)     # [CT, NDT]; i_local = t*CT + p
        d[k * BSH : (k + 1) * BSH] = dp.T.reshape(-1)

    corr = np.zeros(C)
    np.add.at(corr, lab, np.exp(32.0 * d + 3.2))
    tpos = np.zeros(C)
    np.add.at(tpos, lab, np.exp(-32.0 * d + 3.2))

    s_neg = colsum - corr
    s_pos = tpos[lab]
    out = np.log1p(s_neg).mean() + np.log1p(s_pos).mean()
    return np.asarray(out, dtype=np.float32)

